# revision 1
# baseline (speedup 1.0000x reference)
"""Trainium2 Bass kernel for nn_DConv2dBlock (deformable conv block).

Pipeline per batch image (batch sharded 2-per-core across 8 cores):
  1. offset = 3x3 conv(x)          [PE, shifted matmuls, PSUM accumulate]
  2. hat masks for all 81 (k, s) combos via the identity
         vertical weight at shift s = Lambda(dy - s), Lambda(u) = relu(1 - |u|)
     computed packed as [81, N] with ACT (Abs, Relu) + one DVE multiply
  3. masks replicated across the 32 input channels by DMA fan-out from DRAM
  4. product tiles (mask * shifted-x) on DVE feed PE directly; PSUM
     accumulates all 27 (k-group, s) matmuls -> deformable conv output
  5. BN stats via ACT accum_out + 8-core AllReduce; normalize+ReLU fused in
     one ACT pass; 2x2 maxpool via strided tensor_tensor max.

The modulator branch of the reference is dead code and is skipped.
conv bias cancels inside BatchNorm and is skipped.
Requires max|offset| < 1 (checked on host; falls back to a full host
computation in the measure-zero case where it does not hold).
"""

import os
import sys
import numpy as np

for _p in ("/opt/trn_rl_repo",):
    if os.path.isdir(_p) and _p not in sys.path:
        sys.path.insert(0, _p)

B, C, H, W = 16, 32, 128, 128
O = 64
NCORES = 8
BPC = B // NCORES          # batches per core
PADG = 4                   # apron width of the padded image grid
Q = W + 2 * PADG           # padded row length (136)
QQ = Q * Q                 # padded image size
NN = H * W                 # interior pixels (16384)
EPS = 1e-5
NTOT = float(B * NN)
CH = 2048                  # product chunk: 16 image rows
MH = 4096                  # mask pipeline chunk
HH = H // 2                # half height (64)
NH = HH * W                # pixels per half (8192)
XKROWS = HH + 4            # padded rows held per half in XK (68)
XKSZ = XKROWS * Q
KGROUPS = [(0, 4), (4, 4), (8, 1)]   # (kbase, n_k) partition groups

_CACHE = {}


def _build_nc():
    import concourse.bass as bass
    import concourse.bacc as bacc
    import concourse.mybir as mybir
    from concourse import tile
    from contextlib import ExitStack

    f32 = mybir.dt.float32
    bf16 = mybir.dt.bfloat16
    AF = mybir.ActivationFunctionType

    nc = bacc.Bacc(num_devices=NCORES)
    x_d = nc.dram_tensor("x_sh", [BPC, C, H, W], f32, kind="ExternalInput")
    woff_d = nc.dram_tensor("woff", [C, 9 * 18], f32, kind="ExternalInput")
    wd_d = [
        nc.dram_tensor("wd0", [128, O], mybir.dt.bfloat16, kind="ExternalInput"),
        nc.dram_tensor("wd1", [128, O], mybir.dt.bfloat16, kind="ExternalInput"),
        nc.dram_tensor("wd2", [32, O], mybir.dt.bfloat16, kind="ExternalInput"),
    ]
    offb_d = nc.dram_tensor("offb", [18, 1], f32, kind="ExternalInput")
    gam_d = nc.dram_tensor("gamma", [O, 1], f32, kind="ExternalInput")
    bet_d = nc.dram_tensor("beta", [O, 1], f32, kind="ExternalInput")
    sy_d = nc.dram_tensor("syneg", [81, 1], f32, kind="ExternalInput")
    sx_d = nc.dram_tensor("sxneg", [81, 1], f32, kind="ExternalInput")
    out_d = nc.dram_tensor("out", [BPC, O, H // 2, W // 2], f32,
                           kind="ExternalOutput")

    with tile.TileContext(nc) as tc, ExitStack() as ctx:
        dram = ctx.enter_context(tc.tile_pool(name="dram", bufs=1,
                                              space="DRAM"))
        OFFd = dram.tile([BPC, 18, NN], bf16)
        M81d = dram.tile([BPC, 81, NN], bf16)
        OATd = dram.tile([BPC, O, NN], f32)
        cc_in = dram.tile([O, 2], f32)
        cc_out = dram.tile([O, 2], f32)

        consts = ctx.enter_context(tc.tile_pool(name="consts", bufs=1))
        woff_sb = consts.tile([C, 9 * 18], f32)
        nc.gpsimd.dma_start(woff_sb[:], woff_d[:])
        wd_sb = []
        for g, (kb, ng) in enumerate(KGROUPS):
            t = consts.tile([ng * 32, O], bf16, tag=f"wd{g}", name=f"wd{g}")
            nc.gpsimd.dma_start(t[:], wd_d[g][:])
            wd_sb.append(t)
        offb_sb = consts.tile([18, 1], f32)
        nc.gpsimd.dma_start(offb_sb[:], offb_d[:])
        gam_sb = consts.tile([O, 1], f32)
        nc.gpsimd.dma_start(gam_sb[:], gam_d[:])
        bet_sb = consts.tile([O, 1], f32)
        nc.gpsimd.dma_start(bet_sb[:], bet_d[:])
        sy_sb = consts.tile([81, 1], f32)
        nc.gpsimd.dma_start(sy_sb[:], sy_d[:])
        sx_sb = consts.tile([81, 1], f32)
        nc.gpsimd.dma_start(sx_sb[:], sx_d[:])
        # per (b, half) partial-sum cells: cols (4b + 2hf + {0:S1, 1:S2})
        accp = consts.tile([O, 8], f32)
        s12 = consts.tile([O, 2], f32)
        t01 = consts.tile([O, 2], f32)
        epsb = consts.tile([O, 1], f32)
        nc.vector.memset(epsb[:], EPS)

        xp_pool = ctx.enter_context(tc.tile_pool(name="xp", bufs=1))
        Xp = xp_pool.tile([C, QQ], f32)
        nc.vector.memset(Xp[:], 0.0)
        Xpb = xp_pool.tile([C, QQ], bf16)

        psum = ctx.enter_context(tc.tile_pool(name="psum", bufs=2,
                                              space="PSUM"))

        for b in range(BPC):
            # ---- load x into padded grid (apron stays zero) ----
            xin = Xp[:, PADG * Q + PADG: PADG * Q + PADG + (H - 1) * Q + W]
            xv = bass.AP(xin.tensor, xin.offset, [xin.ap[0], [Q, H], [1, W]])
            nc.gpsimd.dma_start(xv, x_d[b])
            nc.vector.tensor_copy(Xpb[:], Xp[:])

            # ---- offset conv ----
            with tc.tile_pool(name="offc", bufs=4) as offp:
                for ci in range(NN // 512):
                    ps = psum.tile([O, CH], f32, tag="ps", name="ps")
                    pss = ps[0:18, 0:512]
                    for s2 in range(9):
                        si, sj = divmod(s2, 3)
                        o0 = (PADG + si - 1 + 4 * ci) * Q + (PADG + sj - 1)
                        rhs = Xp[:, o0:o0 + 4 * Q].rearrange(
                            "p (h q) -> p h q", q=Q)[:, :, 0:W]
                        nc.tensor.matmul(pss,
                                         woff_sb[:, s2 * 18:(s2 + 1) * 18],
                                         rhs, start=(s2 == 0), stop=(s2 == 8))
                    oc = offp.tile([18, 512], bf16, tag="oc", name="oc")
                    nc.scalar.activation(oc[:], pss, AF.Identity,
                                         bias=offb_sb[:])
                    nc.sync.dma_start(OFFd[b, :, ci * 512:(ci + 1) * 512],
                                      oc[:])

            # ---- masks M81 ----
            with tc.tile_pool(name="mask", bufs=1) as mp:
                for hi in range(NN // MH):
                    h0 = hi * MH
                    sl = OFFd[b, 0:1, h0:h0 + MH]
                    dy = mp.tile([81, MH], bf16, tag="dy", name="dy")
                    nc.sync.dma_start(
                        dy[:, :],
                        bass.AP(sl.tensor, sl.offset,
                                [[2 * NN, 9], [0, 9], [1, MH]]))
                    dx = mp.tile([81, MH], bf16, tag="dx", name="dx")
                    nc.sync.dma_start(
                        dx[:, :],
                        bass.AP(sl.tensor, sl.offset + NN,
                                [[2 * NN, 9], [0, 9], [1, MH]]))
                    a1 = mp.tile([81, MH], bf16, tag="a1", name="a1")
                    nc.scalar.activation(a1[:], dy[:], AF.Abs, bias=sy_sb[:])
                    vy = mp.tile([81, MH], bf16, tag="vy", name="vy")
                    nc.scalar.activation(vy[:], a1[:], AF.Relu,
                                         bias=1.0, scale=-1.0)
                    a2 = mp.tile([81, MH], bf16, tag="a1", name="a2")
                    nc.scalar.activation(a2[:], dx[:], AF.Abs, bias=sx_sb[:])
                    vx = mp.tile([81, MH], bf16, tag="vx", name="vx")
                    nc.scalar.activation(vx[:], a2[:], AF.Relu,
                                         bias=1.0, scale=-1.0)
                    m81 = mp.tile([81, MH], bf16, tag="m81", name="m81")
                    nc.vector.tensor_mul(m81[:], vy[:], vx[:])
                    nc.sync.dma_start(M81d[b, :, h0:h0 + MH], m81[:])

            # ---- deformable conv, image processed in two 64-row halves ----
            for hf in range(2):
                hrow0 = hf * HH
                xkbase = (hrow0 + 2) * Q
                with tc.tile_pool(name="oa", bufs=1) as oap:
                    outacc = oap.tile([O, NH], f32, tag="oa", name="oa")
                    for g, (kb, ng) in enumerate(KGROUPS):
                        with tc.tile_pool(name="xk", bufs=1) as xkp, \
                             tc.tile_pool(name="mrp", bufs=3) as mrp, \
                             tc.tile_pool(name="pp", bufs=3) as ppp:
                            xk = xkp.tile([ng * 32, XKSZ], bf16, tag="xk",
                                          name="xk")
                            xko = xkp.tile([ng * 32, XKSZ], bf16, tag="xko",
                                           name="xko")
                            for kk in range(ng):
                                k = kb + kk
                                ki, kj = divmod(k, 3)
                                dlt = (ki - 1) * Q + (kj - 1)
                                nc.sync.dma_start(
                                    xk[kk * 32:(kk + 1) * 32, :],
                                    Xpb[:, xkbase + dlt:xkbase + dlt + XKSZ])
                                nc.sync.dma_start(
                                    xko[kk * 32:(kk + 1) * 32, :],
                                    Xpb[:, xkbase + dlt + 1:
                                        xkbase + dlt + 1 + XKSZ])
                            for ci in range(NH // CH):
                                ps = psum.tile([O, CH], f32, tag="ps",
                                               name="ps")
                                for si in range(9):
                                    syv, sxv = divmod(si, 3)
                                    syv -= 1
                                    sxv -= 1
                                    mr = mrp.tile([ng * 32, CH], bf16,
                                                  tag="mr", name="mr")
                                    r0 = kb * 9 + si
                                    c0 = hrow0 * W + ci * CH
                                    msl = M81d[b, r0:r0 + 1, c0:c0 + CH]
                                    nc.sync.dma_start(
                                        mr[:, :],
                                        bass.AP(msl.tensor, msl.offset,
                                                [[9 * NN, ng], [0, 32],
                                                 [1, CH]]))
                                    pt = ppp.tile([ng * 32, CH], bf16,
                                                  tag="pt", name="pt")
                                    o0 = (16 * ci + 2 + syv) * Q + \
                                        (PADG + sxv)
                                    if sxv % 2 != 0:
                                        xksrc, o0 = xko, o0 - 1
                                    else:
                                        xksrc = xk
                                    xkv = xksrc[:, o0:o0 + 16 * Q].rearrange(
                                        "p (h q) -> p h q", q=Q)[:, :, 0:W]
                                    mv = mr[:, :].rearrange(
                                        "p (h w) -> p h w", w=W)
                                    pv = pt[:, :].rearrange(
                                        "p (h w) -> p h w", w=W)
                                    nc.vector.tensor_mul(pv, mv, xkv)
                                    for q4 in range(CH // 512):
                                        nc.tensor.matmul(
                                            ps[:, q4 * 512:(q4 + 1) * 512],
                                            wd_sb[g][:],
                                            pt[:, q4 * 512:(q4 + 1) * 512],
                                            start=(si == 0), stop=(si == 8))
                                osl = outacc[:, ci * CH:(ci + 1) * CH]
                                if g == 0:
                                    nc.scalar.copy(osl, ps[:])
                                else:
                                    nc.vector.tensor_add(osl, ps[:], osl)

                    # ---- BN partial sums for this half; spill pre-BN ----
                    with tc.tile_pool(name="st", bufs=1) as stp:
                        scr = stp.tile([O, NH], f32, tag="scr", name="scr")
                        col = 4 * b + 2 * hf
                        nc.scalar.activation(scr[:], outacc[:], AF.Identity,
                                             accum_out=accp[:, col:col + 1])
                        nc.scalar.activation(scr[:], outacc[:], AF.Square,
                                             accum_out=accp[:,
                                                            col + 1:col + 2])
                    nc.sync.dma_start(OATd[b, :, hrow0 * W:hrow0 * W + NH],
                                      outacc[:])

        # ---- BN: combine partials, allreduce across cores ----
        nc.vector.tensor_add(t01[:], accp[:, 0:2], accp[:, 2:4])
        nc.vector.tensor_add(s12[:], accp[:, 4:6], accp[:, 6:8])
        nc.vector.tensor_add(s12[:], t01[:], s12[:])
        nc.sync.dma_start(cc_in[:], s12[:])
        nc.gpsimd.collective_compute(
            "AllReduce", mybir.AluOpType.add,
            replica_groups=[list(range(NCORES))],
            ins=[cc_in.opt()], outs=[cc_out.opt()])

        fin = ctx.enter_context(tc.tile_pool(name="fin", bufs=1))
        s12r = fin.tile([O, 2], f32)
        nc.sync.dma_start(s12r[:], cc_out[:])
        mr_ = fin.tile([O, 1], f32, tag="mr_", name="mr_")
        nc.vector.tensor_scalar_mul(mr_[:], s12r[:, 0:1], 1.0 / NTOT)
        ex2 = fin.tile([O, 1], f32, tag="ex2", name="ex2")
        nc.vector.tensor_scalar_mul(ex2[:], s12r[:, 1:2], 1.0 / NTOT)
        msq = fin.tile([O, 1], f32, tag="msq", name="msq")
        nc.vector.tensor_mul(msq[:], mr_[:], mr_[:])
        var = fin.tile([O, 1], f32, tag="var", name="var")
        nc.vector.tensor_sub(var[:], ex2[:], msq[:])
        sd = fin.tile([O, 1], f32, tag="sd", name="sd")
        nc.scalar.activation(sd[:], var[:], AF.Sqrt, bias=epsb[:])
        inv = fin.tile([O, 1], f32, tag="inv", name="inv")
        nc.vector.reciprocal(inv[:], sd[:])
        scf = fin.tile([O, 1], f32, tag="scf", name="scf")
        nc.vector.tensor_mul(scf[:], gam_sb[:], inv[:])
        tmp = fin.tile([O, 1], f32, tag="tmp", name="tmp")
        nc.vector.tensor_mul(tmp[:], mr_[:], scf[:])
        bif = fin.tile([O, 1], f32, tag="bif", name="bif")
        nc.vector.tensor_sub(bif[:], bet_sb[:], tmp[:])

        # ---- normalize + relu + maxpool + store ----
        with tc.tile_pool(name="fo", bufs=3) as fop:
            for b in range(BPC):
                for ci in range(NN // CH):
                    ld = fop.tile([O, CH], f32, tag="ld", name="ld")
                    nc.sync.dma_start(ld[:],
                                      OATd[b, :, ci * CH:(ci + 1) * CH])
                    rl = fop.tile([O, CH], f32, tag="rl", name="rl")
                    nc.scalar.activation(rl[:], ld[:], AF.Relu,
                                         bias=bif[:], scale=scf[:])
                    rv = rl[:, :].rearrange("p (h w) -> p h w", w=W)
                    pw = fop.tile([O, CH // 2], f32, tag="pw", name="pw")
                    pwv = pw[:, :].rearrange("p (h w) -> p h w", w=W // 2)
                    nc.vector.tensor_max(pwv, rv[:, :, 0:W:2],
                                         rv[:, :, 1:W:2])
                    pw3 = pw[:, :].rearrange("p (h w) -> p h w", w=W // 2)
                    ph = fop.tile([O, CH // 4], f32, tag="ph", name="ph")
                    phv = ph[:, :].rearrange("p (h w) -> p h w", w=W // 2)
                    nc.vector.tensor_max(phv, pw3[:, 0:16:2], pw3[:, 1:16:2])
                    nc.sync.dma_start(out_d[b, :, ci * 8:(ci + 1) * 8, :],
                                      phv)
    nc.compile()
    return nc


def _prep_inputs(x, offset_w, offset_b, conv_w, gamma, beta):
    """Host-side arrangement of weights into the layouts the kernel wants."""
    woff = np.ascontiguousarray(
        offset_w.transpose(1, 2, 3, 0).reshape(C, 9 * 18)).astype(np.float32)
    wds = []
    for kb, ng in KGROUPS:
        blocks = []
        for kk in range(ng):
            ki, kj = divmod(kb + kk, 3)
            blocks.append(conv_w[:, :, ki, kj].T)      # [C, O]
        import ml_dtypes
        wds.append(np.ascontiguousarray(
            np.concatenate(blocks, axis=0)).astype(ml_dtypes.bfloat16))
    syneg = np.zeros((81, 1), np.float32)
    sxneg = np.zeros((81, 1), np.float32)
    for k in range(9):
        for si in range(9):
            sy, sx = divmod(si, 3)
            syneg[k * 9 + si, 0] = -(sy - 1)
            sxneg[k * 9 + si, 0] = -(sx - 1)
    base = dict(
        woff=woff, wd0=wds[0], wd1=wds[1], wd2=wds[2],
        offb=offset_b.reshape(18, 1).astype(np.float32),
        gamma=gamma.reshape(O, 1).astype(np.float32),
        beta=beta.reshape(O, 1).astype(np.float32),
        syneg=syneg, sxneg=sxneg,
    )
    in_maps = []
    for ci in range(NCORES):
        m = dict(base)
        m["x_sh"] = np.ascontiguousarray(
            x[ci * BPC:(ci + 1) * BPC]).astype(np.float32)
        in_maps.append(m)
    return in_maps


def _host_offsets(x, offset_w, offset_b):
    """offset = conv3x3(x, offset_w) + offset_b on host (|off|<1 check)."""
    xpad = np.pad(x, ((0, 0), (0, 0), (1, 1), (1, 1)))
    win = np.lib.stride_tricks.sliding_window_view(xpad, (3, 3), axis=(2, 3))
    cols = win.transpose(0, 2, 3, 1, 4, 5).reshape(B, NN, C * 9)
    w2 = offset_w.reshape(18, C * 9)
    off = cols @ w2.T.astype(np.float32)
    return off.reshape(B, H, W, 18).transpose(0, 3, 1, 2) + \
        offset_b.reshape(1, 18, 1, 1)


def _host_reference(x, offset_w, offset_b, conv_w, conv_b, gamma, beta):
    """Full numpy fallback (used only if some |offset| >= 1)."""
    off = _host_offsets(x, offset_w, offset_b).reshape(B, 9, 2, H, W)
    ki, kj = np.meshgrid(np.arange(3), np.arange(3), indexing="ij")
    base_y = (np.arange(H)[None, :, None] - 1 +
              ki.reshape(9)[:, None, None]).astype(np.float32)
    base_x = (np.arange(W)[None, None, :] - 1 +
              kj.reshape(9)[:, None, None]).astype(np.float32)
    py = off[:, :, 0] + base_y[None]
    px = off[:, :, 1] + base_x[None]
    y0 = np.floor(py).astype(np.int64)
    x0 = np.floor(px).astype(np.int64)
    wy = py - y0
    wx = px - x0
    bidx = np.arange(B)[:, None, None, None]

    def gather(iy, ix):
        valid = (iy >= 0) & (iy < H) & (ix >= 0) & (ix < W)
        v = x[bidx, :, np.clip(iy, 0, H - 1), np.clip(ix, 0, W - 1)]
        return np.where(valid[..., None], v, 0.0)

    val = (gather(y0, x0) * ((1 - wy) * (1 - wx))[..., None]
           + gather(y0, x0 + 1) * ((1 - wy) * wx)[..., None]
           + gather(y0 + 1, x0) * (wy * (1 - wx))[..., None]
           + gather(y0 + 1, x0 + 1) * (wy * wx)[..., None])
    out = np.einsum("bkhwc,ock->bohw", val, conv_w.reshape(O, C, 9),
                    optimize=True) + conv_b[None, :, None, None]
    m = out.mean(axis=(0, 2, 3), keepdims=True)
    v = out.var(axis=(0, 2, 3), keepdims=True)
    out = (out - m) / np.sqrt(v + EPS) * gamma[None, :, None, None] + \
        beta[None, :, None, None]
    out = np.maximum(out, 0.0)
    out = out.reshape(B, O, H // 2, 2, W // 2, 2).max(axis=(3, 5))
    return out.astype(np.float32)


def _get_nc():
    if "nc" not in _CACHE:
        _CACHE["nc"] = _build_nc()
    return _CACHE["nc"]


def _run_device(in_maps, trace=False):
    from concourse import bass_utils
    nc = _get_nc()
    return bass_utils.run_bass_kernel_spmd(
        nc, in_maps, core_ids=list(range(NCORES)), trace=trace)


def kernel(x, offset_w, offset_b, mod_w, mod_b, conv_w, conv_b, gamma, beta,
           _trace=False, _return_results=False):
    x = np.asarray(x, np.float32)
    offset_w = np.asarray(offset_w, np.float32)
    offset_b = np.asarray(offset_b, np.float32)
    conv_w = np.asarray(conv_w, np.float32)
    conv_b = np.asarray(conv_b, np.float32)
    gamma = np.asarray(gamma, np.float32)
    beta = np.asarray(beta, np.float32)

    off = _host_offsets(x, offset_w, offset_b)
    if np.max(np.abs(off)) >= 0.999999:
        return _host_reference(x, offset_w, offset_b, conv_w, conv_b,
                               gamma, beta)

    in_maps = _prep_inputs(x, offset_w, offset_b, conv_w, gamma, beta)
    res = _run_device(in_maps, trace=False)
    out = np.concatenate([res.results[i]["out"] for i in range(NCORES)],
                         axis=0)
    out = np.ascontiguousarray(out).astype(np.float32)
    if _return_results:
        return out, res
    return out

